# revision 26
# baseline (speedup 1.0000x reference)
"""Trainium2 Bass kernel for nn_Attention4D_77644418777285.

Attention4D block (EfficientViT-style): 1x1-conv QKV + BN, depthwise-3x3
local-V branch, relative-position bias, talking-heads attention (8 heads,
49 tokens), projection. Batch 512 sharded 64-per-core across 8 NeuronCores
(pure data parallel; weights replicated).

v3 layout (per core, 64 images, groups of 8 images):
  - x arrives channel-major from host ([3,128,NT] bf16); output returned
    channel-major bf16 and re-transposed on host — no PE transposes.
  - Key-token axis (m) padded to 64 slots per image (8x8 grid, shared
    guard row/col, one-time zeroed): every shifted view is a plain 2-D
    column offset, v token-major tiles come from a single [128,128]
    PE transpose-matmul per (image-pair, channel-tile), and the attention
    middle (logits rows, talking-head blocks, selector, bias) runs in the
    64-slot row space with zero weights on guard slots.
  - depthwise 3x3 conv split between DVE (scalar_tensor_tensor taps into
    a B2-seeded SBUF accumulator) and PE (per-channel diagonal-matrix
    matmuls accumulating shifted windows in PSUM).
  - talking-heads + rel-pos bias injected pre-exp into the th1 psum chain;
    softmax denominators via selector matmul, batched DVE reciprocal,
    normalization broadcast via constant delta matmul.
  - assembly o + v_local(+psum part) + relu, projection, straight DMA out.
"""

import numpy as np
import ml_dtypes

R = 7
N = 49
H = 8
KD = 32
D = 128
DH = 1024
DIM = 384
SCALE = KD ** -0.5
NCORES = 8
B_FULL = 512
GW = 8 * N          # 392 compact cols per group of 8 images
SL = 64             # padded slots per image (8x8)
GWP = 8 * SL        # 512 padded cols per group
PADG = 16           # guard cols at each end of padded tiles

# conv tap split: (dy, dx) lists
TAPS_DVE = [(0, 0), (0, -1), (0, 1)]
TAPS_PE = [(-1, -1), (-1, 0), (-1, 1), (1, -1), (1, 0), (1, 1)]

_BF16 = ml_dtypes.bfloat16


def _bias_idxs(r):
    pos = np.stack(np.meshgrid(np.arange(r), np.arange(r))).reshape(2, -1)
    rel = np.abs(pos[:, :, None] - pos[:, None, :])
    return (rel[0] * r + rel[1]).reshape(-1)


def _slot(t):
    """compact token t (0..48) -> padded slot (0..63)."""
    return (t // 7) * 8 + (t % 7)


_SLOTS = np.array([_slot(t) for t in range(N)])


def make_consts(inp):
    f32 = np.float32
    g = {k: np.asarray(v, f32) for k, v in inp.items()}

    th1, th1_b = g['th1_w'], g['th1_b']
    th2, th2_b = g['th2_w'], g['th2_b']

    W_q = g['q_w'] * g['q_g'][None, :] * SCALE
    b_q = (g['q_b'] * g['q_g'] + g['q_beta']) * SCALE
    W_k = g['k_w'] * g['k_g'][None, :]
    b_k = g['k_b'] * g['k_g'] + g['k_beta']
    W_v = g['v_w'] * g['v_g'][None, :]
    b_v = g['v_b'] * g['v_g'] + g['v_beta']

    idxs = _bias_idxs(R)
    bias_full = g['attn_bias'][:, idxs].reshape(H, N, N)          # [h, n, m]
    biasp = np.einsum('hg,hnm->gnm', th1, bias_full) + th1_b[:, None, None]

    w9 = g['vl_w'].reshape(9, DH)                                  # [tap, c]
    w_eff = (w9 * g['vl_g'][None, :]).astype(f32)                  # [tap, c]
    sumw = np.zeros((DH, N), f32)
    for t in range(9):
        dy, dx = t // 3 - 1, t % 3 - 1
        for s in range(N):
            y, x = s // 7, s % 7
            if 0 <= y + dy < 7 and 0 <= x + dx < 7:
                sumw[:, s] += w9[t]
    s2 = th2.sum(axis=0) + N * th2_b                               # [g]
    B2 = (g['vl_g'][:, None] * (b_v[:, None] * sumw + g['vl_b'][:, None])
          + g['vl_beta'][:, None]
          + (b_v * s2[np.repeat(np.arange(H), D)])[:, None])       # [c, s=49]

    W_p = g['proj_w'] * g['proj_g'][None, :]
    b_p = g['proj_b'] * g['proj_g'] + g['proj_beta']

    consts = {}
    wqk = np.concatenate([W_q, W_k], axis=1).reshape(3, 128, 512)
    consts['wqk'] = wqk.astype(_BF16)
    consts['wv'] = W_v.reshape(3, 128, DH).astype(_BF16)
    consts['wp'] = W_p.reshape(8, 128, DIM).astype(_BF16)
    consts['bqk'] = np.concatenate([b_q, b_k]).reshape(4, 128).astype(f32)
    consts['bp'] = b_p.reshape(3, 128).astype(f32)

    # Talking heads as [jo, ji, K=128, M=128] block matrices in the
    # (hh, slot64) row space: row (hh*64 + slot(m)) of input tile ji =
    # head (2*ji+hh), key m; col likewise for output tile jo.
    def th_blocks(thw):
        Wb = np.zeros((4, 4, 128, 128), f32)
        eye = np.zeros((SL, SL), f32)
        eye[_SLOTS, _SLOTS] = 1.0
        for jo in range(4):
            for ji in range(4):
                for hhi in range(2):
                    for hho in range(2):
                        c = thw[2 * ji + hhi, 2 * jo + hho]
                        Wb[jo, ji, hhi * 64:hhi * 64 + SL,
                           hho * 64:hho * 64 + SL] += c * eye
        return Wb
    consts['w1s'] = th_blocks(th1).astype(_BF16)
    consts['w2s'] = th_blocks(th2).astype(_BF16)

    sel = np.zeros((128, 2), f32)
    sel[_SLOTS, 0] = 1.0
    sel[64 + _SLOTS, 1] = 1.0
    consts['sel'] = sel.astype(_BF16)

    dlt = np.zeros((128, 128), f32)
    for j in range(4):
        dlt[32 * j + 0, 0:64] = 1.0
        dlt[32 * j + 1, 64:128] = 1.0
    consts['dlt'] = dlt.astype(f32)

    # th1-transformed rel-pos bias in [(hh, slot) x (img, n)] rows,
    # replicated over 8 images (pre-exp; injected into the th1 psum chain).
    bsb = np.zeros((4, 128, GW), f32)
    for j in range(4):
        for hh in range(2):
            b = biasp[2 * j + hh].T                                # [m, n]
            bsb[j, hh * 64 + _SLOTS] = np.tile(b, (1, 8))
    consts['biasp'] = bsb.astype(_BF16)

    # DVE tap weights: sbuf [128, 8, 9] (c-part, ct, tap)
    consts['w9t'] = w_eff.reshape(9, 8, 128).transpose(2, 1, 0).copy().astype(f32)

    # PE tap diagonal weights: [8 ct, n_pe, 128, 128]
    dw = np.zeros((8, len(TAPS_PE), 128, 128), f32)
    for ct in range(8):
        for ti, (dy, dx) in enumerate(TAPS_PE):
            tap = (dy + 1) * 3 + (dx + 1)
            np.fill_diagonal(dw[ct, ti], w_eff[tap, ct * 128:(ct + 1) * 128])
    consts['dw'] = dw.astype(_BF16)

    # B2 in padded-slot layout: [8, 128, 64] (zeros at guard slots)
    b2p = np.zeros((8, 128, SL), f32)
    b2p[:, :, _SLOTS] = B2.reshape(8, 128, N)
    consts['b2p'] = b2p.astype(_BF16)

    consts['ident'] = np.eye(128, dtype=f32).astype(_BF16)
    return consts


def build_program(n_imgs, loop_n=1, stage=9):
    """Build the Bass program for one core processing n_imgs images.

    loop_n > 1 wraps the whole compute (including I/O DMA) in a hardware
    loop — used only by the timing harness to measure per-iteration HW time.
    """
    from contextlib import ExitStack
    import concourse.bass as bass
    import concourse.tile as tile
    from concourse import bacc, mybir

    f32 = mybir.dt.float32
    bf16 = mybir.dt.bfloat16
    AF = mybir.ActivationFunctionType
    ALU = mybir.AluOpType

    NI = n_imgs
    NG = NI // 8                 # groups of 8 images
    NT = NI * N                  # tokens

    nc = bacc.Bacc("TRN2", target_bir_lowering=False, debug=False,
                   enable_asserts=False)

    x_d = nc.dram_tensor("x", [3, 128, NT], bf16, kind="ExternalInput").ap()
    wqk_d = nc.dram_tensor("wqk", [3, 128, 512], bf16, kind="ExternalInput").ap()
    wv_d = nc.dram_tensor("wv", [3, 128, DH], bf16, kind="ExternalInput").ap()
    wp_d = nc.dram_tensor("wp", [8, 128, DIM], bf16, kind="ExternalInput").ap()
    bqk_d = nc.dram_tensor("bqk", [4, 128], f32, kind="ExternalInput").ap()
    bp_d = nc.dram_tensor("bp", [3, 128], f32, kind="ExternalInput").ap()
    w1_d = nc.dram_tensor("w1s", [4, 4, 128, 128], bf16, kind="ExternalInput").ap()
    w2_d = nc.dram_tensor("w2s", [4, 4, 128, 128], bf16, kind="ExternalInput").ap()
    sel_d = nc.dram_tensor("sel", [128, 2], bf16, kind="ExternalInput").ap()
    dlt_d = nc.dram_tensor("dlt", [128, 128], f32, kind="ExternalInput").ap()
    bias_d = nc.dram_tensor("biasp", [4, 128, GW], bf16, kind="ExternalInput").ap()
    w9_d = nc.dram_tensor("w9t", [128, 8, 9], f32, kind="ExternalInput").ap()
    dw_d = nc.dram_tensor("dw", [8, len(TAPS_PE), 128, 128], bf16,
                          kind="ExternalInput").ap()
    b2_d = nc.dram_tensor("b2p", [8, 128, SL], bf16, kind="ExternalInput").ap()
    id_d = nc.dram_tensor("ident", [128, 128], bf16, kind="ExternalInput").ap()
    out_d = nc.dram_tensor("out", [3, 128, NT], bf16, kind="ExternalOutput").ap()

    with tile.TileContext(nc) as tc, ExitStack() as ctx:
        const = ctx.enter_context(tc.tile_pool(name="const", bufs=1))
        pers = ctx.enter_context(tc.tile_pool(name="pers", bufs=1))
        mid = ctx.enter_context(tc.tile_pool(name="mid", bufs=6))
        accp = ctx.enter_context(tc.tile_pool(name="accp", bufs=1))
        stg = ctx.enter_context(tc.tile_pool(name="stg", bufs=3))
        ps = ctx.enter_context(tc.tile_pool(name="ps", bufs=8, space="PSUM"))

        dma = nc.sync.dma_start

        # ---------------- constants ----------------
        wqk_t = [const.tile([128, 512], bf16, name=f"wqk{k}", tag=f"wqk{k}") for k in range(3)]
        wv_t = [const.tile([128, DH], bf16, name=f"wv{k}", tag=f"wv{k}") for k in range(3)]
        wp_t = [const.tile([128, DIM], bf16, name=f"wp{k}", tag=f"wp{k}") for k in range(8)]
        for k in range(3):
            dma(out=wqk_t[k], in_=wqk_d[k])
            dma(out=wv_t[k], in_=wv_d[k])
        for k in range(8):
            dma(out=wp_t[k], in_=wp_d[k])
        bqk_t = const.tile([128, 4], f32, name="bqk", tag="bqk")
        dma(out=bqk_t, in_=bass.AP(tensor=bqk_d.tensor, offset=0,
                                   ap=[[1, 128], [128, 4]]))
        bp_t = const.tile([128, 3], f32, name="bp", tag="bp")
        dma(out=bp_t, in_=bass.AP(tensor=bp_d.tensor, offset=0,
                                  ap=[[1, 128], [128, 3]]))
        w1_t = const.tile([128, 16, 128], bf16, name="w1", tag="w1")
        dma(out=w1_t, in_=bass.AP(tensor=w1_d.tensor, offset=0,
                                  ap=[[128, 128], [128 * 128, 16], [1, 128]]))
        w2_t = const.tile([128, 16, 128], bf16, name="w2", tag="w2")
        dma(out=w2_t, in_=bass.AP(tensor=w2_d.tensor, offset=0,
                                  ap=[[128, 128], [128 * 128, 16], [1, 128]]))
        sel_t = const.tile([128, 2], bf16, name="sel", tag="sel")
        dma(out=sel_t, in_=sel_d)
        dlt_t = const.tile([128, 128], f32, name="dlt", tag="dlt")
        dma(out=dlt_t, in_=dlt_d)
        bias_t = [const.tile([128, GW], bf16, name=f"bi{j}", tag=f"bi{j}") for j in range(4)]
        for j in range(4):
            dma(out=bias_t[j], in_=bias_d[j])
        w9_t = const.tile([128, 8, 9], f32, name="w9", tag="w9")
        dma(out=w9_t, in_=w9_d)
        dw_t = const.tile([128, 8 * len(TAPS_PE), 128], bf16, name="dw", tag="dw")
        dma(out=dw_t, in_=bass.AP(tensor=dw_d.tensor, offset=0,
                                  ap=[[128, 128], [128 * 128, 8 * len(TAPS_PE)],
                                      [1, 128]]))
        id_t = const.tile([128, 128], bf16, name="id", tag="id")
        dma(out=id_t, in_=id_d)

        # ---------------- persistent / slot tiles ----------------
        xg = [[pers.tile([128, GW], bf16, name=f"xg{k}_{s}", tag=f"xg{k}_{s}")
               for s in range(2)] for k in range(3)]
        qcm = [[pers.tile([128, GW], bf16, name=f"q{t}_{s}", tag=f"q{t}_{s}")
                for s in range(2)] for t in range(2)]
        kcm = [[pers.tile([128, GWP], bf16, name=f"k{t}_{s}", tag=f"k{t}_{s}")
                for s in range(2)] for t in range(2)]
        vcm = [[pers.tile([128, GWP + 2 * PADG], bf16, name=f"vc{c}_{s}",
                          tag=f"vc{c}_{s}")
                for s in range(2)] for c in range(8)]
        vtokE = [pers.tile([64, DH], bf16, name=f"vtE{s}", tag=f"vtE{s}") for s in range(8)]
        vtokO = [pers.tile([64, DH], bf16, name=f"vtO{s}", tag=f"vtO{s}") for s in range(8)]
        Ls = [[pers.tile([128, GW], bf16, name=f"Ls{j}_{s}", tag=f"Ls{j}_{s}") for s in range(2)]
              for j in range(4)]
        a2h = [[[pers.tile([64, GW], bf16, name=f"a2_{j}_{hh}_{s}",
                            tag=f"a2_{j}_{hh}_{s}") for s in range(2)]
                 for hh in range(2)] for j in range(4)]
        r_sb = [pers.tile([128, GW], f32, name=f"rsb{s}", tag=f"rsb{s}") for s in range(2)]

        # one-time zero init: padded tiles fully (guard slots must stay 0)
        for c in range(8):
            for s in range(2):
                nc.vector.memset(vcm[c][s], 0.0)
        for t in range(2):
            for s in range(2):
                nc.vector.memset(kcm[t][s], 0.0)

        # 4-d views of a padded group region: [p, i, y(7), x(7)] valid slots
        def padview(tile_, base):
            v = tile_[:, base:base + GWP].rearrange("p (i q) -> p i q", q=SL)
            v = v.rearrange("p i (y x) -> p i y x", x=8)
            return v[:, :, 0:7, 0:7]

        def cview(tile_):
            return tile_.rearrange("p (i y x) -> p i y x", y=7, x=7)

        def group_body(g):
            sl = g % 2          # phase slot
            c0 = g * GW

            # --- x load (channel-major direct) ---
            for kt in range(3):
                dma(out=xg[kt][sl], in_=x_d[kt][:, c0:c0 + GW])

            # --- QKV channel-major ---
            for mt in range(12):
                qp = ps.tile([128, 512], f32, name="ps", tag="ps")
                for kt in range(3):
                    if mt < 4:
                        w = wqk_t[kt][:, mt * 128:(mt + 1) * 128]
                    else:
                        w = wv_t[kt][:, (mt - 4) * 128:(mt - 3) * 128]
                    nc.tensor.matmul(qp[:, 0:GW], w,
                                     xg[kt][sl],
                                     start=(kt == 0), stop=(kt == 2))
                if mt < 2:
                    nc.scalar.activation(qcm[mt][sl], qp[:, 0:GW],
                                         AF.Identity,
                                         bias=bqk_t[:, mt:mt + 1])
                elif mt < 4:
                    # k with bias, scattered into the padded-slot layout
                    sc = mid.tile([128, GW], bf16, name="ksc", tag="ksc")
                    nc.scalar.activation(sc, qp[:, 0:GW], AF.Identity,
                                         bias=bqk_t[:, mt:mt + 1])
                    nc.vector.tensor_copy(padview(kcm[mt - 2][sl], 0), cview(sc))
                else:
                    nc.vector.tensor_copy(padview(vcm[mt - 4][sl], PADG),
                                          cview(qp[:, 0:GW]))

            # --- v token-major via PE pair-transposes ---
            if stage < 2:
                return
            for pr in range(4):
                p = 4 * g + pr
                vp = [ps.tile([128, 512], f32, name="ps", tag="ps") for _ in range(2)]
                for ct in range(8):
                    nh, cc = ct // 4, (ct % 4) * 128
                    nc.tensor.matmul(
                        vp[nh][:, cc:cc + 128],
                        vcm[ct][sl][:, PADG + pr * 128: PADG + (pr + 1) * 128],
                        id_t, start=True, stop=True)
                nc.vector.tensor_copy(vtokE[p % 8][:, 0:512], vp[0][0:64, :])
                nc.scalar.activation(vtokO[p % 8][:, 0:512],
                                     vp[0][64:128, :], AF.Copy)
                nc.vector.tensor_copy(vtokE[p % 8][:, 512:1024], vp[1][0:64, :])
                nc.scalar.activation(vtokO[p % 8][:, 512:1024],
                                     vp[1][64:128, :], AF.Copy)

            # --- depthwise conv: DVE taps into B2-seeded acc ---
            if stage < 3:
                return
            acc_t = []
            for ct in range(8):
                acc = accp.tile([128, GWP], bf16, name=f"acc{ct}", tag=f"acc{ct}")
                dma(out=acc, in_=bass.AP(tensor=b2_d.tensor,
                                         offset=ct * 128 * SL,
                                         ap=[[SL, 128], [0, 8], [1, SL]]))
                for dy, dx in TAPS_DVE:
                    tap = (dy + 1) * 3 + (dx + 1)
                    dlta = 8 * dy + dx
                    nc.vector.scalar_tensor_tensor(
                        out=acc, in0=vcm[ct][sl][:, PADG + dlta: PADG + dlta + GWP],
                        scalar=w9_t[:, ct, tap:tap + 1],
                        in1=acc, op0=ALU.mult, op1=ALU.add)
                acc_t.append(acc)

            # --- qk logits (rows = (hh, slot64)) ---
            if stage < 4:
                return
            Lp = [ps.tile([128, 512], f32, name="ps", tag="ps") for _ in range(4)]
            for ig in range(8):
                for h in range(H):
                    j, hh = h // 2, h % 2
                    t4, row = h // 4, (h % 4) * 32
                    nc.tensor.matmul(
                        Lp[j][64 * hh: 64 * hh + SL, ig * N:(ig + 1) * N],
                        kcm[t4][sl][row:row + 32, ig * SL:(ig + 1) * SL],
                        qcm[t4][sl][row:row + 32, ig * N:(ig + 1) * N],
                        start=True, stop=True,
                        tile_position=(row, 64 * hh))
            for j in range(4):
                nc.scalar.activation(Ls[j][sl], Lp[j][:, 0:GW], AF.Copy)

            # --- talking heads 1 (+ rel-pos bias) + exp ---
            if stage < 5:
                return
            E = []
            L2p = [ps.tile([128, 512], f32, name="ps", tag="ps") for _ in range(4)]
            for jo in range(4):
                for ji in range(4):
                    nc.tensor.matmul(L2p[jo][:, 0:GW],
                                     w1_t[:, jo * 4 + ji, :],
                                     Ls[ji][sl],
                                     start=(ji == 0), stop=False)
                nc.tensor.matmul(L2p[jo][:, 0:GW], id_t, bias_t[jo],
                                 start=False, stop=True)
            for jo in range(4):
                e = mid.tile([128, GW], bf16, name="E", tag="E")
                nc.scalar.activation(e, L2p[jo][:, 0:GW], AF.Exp)
                E.append(e)

            # --- softmax denominator ---
            if stage < 6:
                return
            csp = ps.tile([128, 512], f32, name="ps", tag="ps")
            for j in range(4):
                nc.tensor.matmul(csp[32 * j: 32 * j + 2, 0:GW], sel_t, E[j],
                                 start=True, stop=True,
                                 tile_position=(0, 32 * j))
            for j in range(4):
                nc.vector.reciprocal(r_sb[sl][32 * j: 32 * j + 2, :],
                                     csp[32 * j: 32 * j + 2, 0:GW])

            # --- normalize + talking heads 2 ---
            A = []
            for j in range(4):
                rp = ps.tile([128, 512], f32, name="ps", tag="ps")
                nc.tensor.matmul(rp[:, 0:GW], dlt_t[32 * j: 32 * j + 2, :],
                                 r_sb[sl][32 * j: 32 * j + 2, :],
                                 start=True, stop=True,
                                 tile_position=(32 * j, 0))
                a = mid.tile([128, GW], bf16, name="A", tag="A")
                nc.vector.tensor_mul(a, E[j], rp[:, 0:GW])
                A.append(a)
            A2p = [ps.tile([128, 512], f32, name="ps", tag="ps") for _ in range(4)]
            for jo in range(4):
                for ji in range(4):
                    nc.tensor.matmul(A2p[jo][:, 0:GW],
                                     w2_t[:, jo * 4 + ji, :],
                                     A[ji],
                                     start=(ji == 0), stop=(ji == 3))
            for jo in range(4):
                nc.scalar.activation(a2h[jo][0][sl], A2p[jo][0:64, 0:GW], AF.Copy)
                nc.scalar.activation(a2h[jo][1][sl], A2p[jo][64:128, 0:GW],
                                     AF.Copy)

            # --- attention * V (+ PE conv taps), assembly, relu ---
            if stage < 7:
                return
            relu_t = []
            for ct in range(8):
                op2 = ps.tile([128, 512], f32, name="ps", tag="ps")
                jo, hh = ct // 2, ct % 2
                for ig in range(8):
                    i = 8 * g + ig
                    pp = ig % 2
                    vt = (vtokE if pp == 0 else vtokO)[(i // 2) % 8]
                    nc.tensor.matmul(
                        op2[:, ig * N:(ig + 1) * N],
                        vt[0:SL, ct * 128:(ct + 1) * 128],
                        a2h[jo][hh][sl][0:SL, ig * N:(ig + 1) * N],
                        start=True, stop=True)
                if stage >= 8:
                    cps = ps.tile([128, 512], f32, name="ps", tag="ps")
                    for ti, (dy, dx) in enumerate(TAPS_PE):
                        dlta = 8 * dy + dx
                        nc.tensor.matmul(
                            cps[:, 0:GWP],
                            dw_t[:, ct * len(TAPS_PE) + ti, :],
                            vcm[ct][sl][:, PADG + dlta: PADG + dlta + GWP],
                            start=(ti == 0), stop=(ti == len(TAPS_PE) - 1))
                tmp = mid.tile([128, GW], bf16, name="tmp", tag="tmp", bufs=3)
                opv = op2[:, 0:GW].rearrange("p (i y x) -> p i y x", y=7, x=7)
                accv = acc_t[ct].rearrange("p (i q) -> p i q", q=SL)
                accv = accv.rearrange("p i (y x) -> p i y x",
                                      x=8)[:, :, 0:7, 0:7]
                nc.vector.tensor_add(cview(tmp), opv, accv)
                if stage >= 8:
                    cpsv = cps[:, 0:GWP].rearrange("p (i q) -> p i q", q=SL)
                    cpsv = cpsv.rearrange("p i (y x) -> p i y x", x=8)[:, :, 0:7, 0:7]
                    nc.vector.tensor_add(cview(tmp), cview(tmp), cpsv)
                rl = mid.tile([128, GW], bf16, name="rl", tag="rl", bufs=10)
                nc.vector.tensor_scalar_max(rl, tmp, 0.0)
                relu_t.append(rl)

            # --- projection + store ---
            for mt in range(3):
                st = stg.tile([128, GW], bf16, name="st", tag="st")
                pp_ = ps.tile([128, 512], f32, name="ps", tag="ps")
                for kt in range(8):
                    nc.tensor.matmul(pp_[:, 0:GW],
                                     wp_t[kt][:, mt * 128:(mt + 1) * 128],
                                     relu_t[kt],
                                     start=(kt == 0), stop=(kt == 7))
                nc.scalar.activation(st, pp_[:, 0:GW], AF.Identity,
                                     bias=bp_t[:, mt:mt + 1])
                dma(out=out_d[mt][:, c0:c0 + GW], in_=st)

        if loop_n > 1:
            with tc.For_i(0, loop_n, 1):
                for g in range(NG):
                    group_body(g)
        else:
            for g in range(NG):
                group_body(g)

    nc.compile()
    return nc


_CACHE = {}


def _get_program(n_imgs):
    if n_imgs not in _CACHE:
        _CACHE[n_imgs] = build_program(n_imgs)
    return _CACHE[n_imgs]


def make_in_maps(inputs, n_cores=NCORES):
    """Host prep: shard + channel-major x, build replicated constants."""
    consts = make_consts(inputs)
    x = np.asarray(inputs['x'], np.float32)
    B = x.shape[0]
    ni = B // n_cores
    nt = ni * N
    x = x.reshape(B, N, DIM)
    in_maps = []
    for c in range(n_cores):
        m = dict(consts)
        xc = x[c * ni:(c + 1) * ni].reshape(nt, DIM).T    # [384, nt]
        m['x'] = np.ascontiguousarray(xc).reshape(3, 128, nt).astype(_BF16)
        in_maps.append(m)
    return in_maps, ni


def assemble_out(results, ni):
    """[3,128,nt] bf16 per core -> full [B, R, R, DIM] f32."""
    nt = ni * N
    outs = []
    for r in results:
        oc = np.asarray(r['out'], np.float32).reshape(DIM, nt)
        outs.append(oc.T.reshape(ni, R, R, DIM))
    return np.concatenate(outs, axis=0)


def kernel(**inputs):
    from concourse import bass_utils
    in_maps, ni = make_in_maps(inputs)
    nc = _get_program(ni)
    res = bass_utils.run_bass_kernel_spmd(
        nc, in_maps, core_ids=list(range(NCORES)))
    return assemble_out(res.results, ni).astype(np.float32)


# revision 38
# speedup vs baseline: 2.0847x; 2.0847x over previous
"""Trainium2 Bass kernel for nn_Attention4D_77644418777285.

Attention4D block (EfficientViT-style): 1x1-conv QKV + BN, depthwise-3x3
local-V branch, relative-position bias, talking-heads attention (8 heads,
49 tokens), projection. Batch 512 sharded 64-per-core across 8 NeuronCores
(pure data parallel; weights replicated).

v3 layout (per core, 64 images, groups of 8 images):
  - x arrives channel-major from host ([3,128,NT] bf16); output returned
    channel-major bf16 and re-transposed on host — no PE transposes.
  - Key-token axis (m) padded to 64 slots per image (8x8 grid, shared
    guard row/col, one-time zeroed): every shifted view is a plain 2-D
    column offset, v token-major tiles come from a single [128,128]
    PE transpose-matmul per (image-pair, channel-tile), and the attention
    middle (logits rows, talking-head blocks, selector, bias) runs in the
    64-slot row space with zero weights on guard slots.
  - depthwise 3x3 conv split between DVE (scalar_tensor_tensor taps into
    a B2-seeded SBUF accumulator) and PE (per-channel diagonal-matrix
    matmuls accumulating shifted windows in PSUM).
  - talking-heads + rel-pos bias injected pre-exp into the th1 psum chain;
    softmax denominators via selector matmul, batched DVE reciprocal,
    normalization broadcast via constant delta matmul.
  - assembly o + v_local(+psum part) + relu, projection, straight DMA out.
"""

import numpy as np
import ml_dtypes

R = 7
N = 49
H = 8
KD = 32
D = 128
DH = 1024
DIM = 384
SCALE = KD ** -0.5
NCORES = 8
B_FULL = 512
GW = 8 * N          # 392 compact cols per group of 8 images
SL = 64             # padded slots per image (8x8)
GWP = 8 * SL        # 512 padded cols per group
PADG = 16           # guard cols at each end of padded tiles

# conv tap split: (dy, dx) lists
TAPS_DVE = [(0, 0), (0, -1), (0, 1)]
TAPS_PE = [(-1, -1), (-1, 0), (-1, 1), (1, -1), (1, 0), (1, 1)]

_BF16 = ml_dtypes.bfloat16
_FP8 = ml_dtypes.float8_e4m3


def _bias_idxs(r):
    pos = np.stack(np.meshgrid(np.arange(r), np.arange(r))).reshape(2, -1)
    rel = np.abs(pos[:, :, None] - pos[:, None, :])
    return (rel[0] * r + rel[1]).reshape(-1)


def _slot(t):
    """compact token t (0..48) -> padded slot (0..63)."""
    return (t // 7) * 8 + (t % 7)


_SLOTS = np.array([_slot(t) for t in range(N)])


def make_consts(inp):
    f32 = np.float32
    g = {k: np.asarray(v, f32) for k, v in inp.items()}

    th1, th1_b = g['th1_w'], g['th1_b']
    th2, th2_b = g['th2_w'], g['th2_b']

    W_q = g['q_w'] * g['q_g'][None, :] * SCALE
    b_q = (g['q_b'] * g['q_g'] + g['q_beta']) * SCALE
    W_k = g['k_w'] * g['k_g'][None, :]
    b_k = g['k_b'] * g['k_g'] + g['k_beta']
    W_v = g['v_w'] * g['v_g'][None, :]
    b_v = g['v_b'] * g['v_g'] + g['v_beta']

    idxs = _bias_idxs(R)
    bias_full = g['attn_bias'][:, idxs].reshape(H, N, N)          # [h, n, m]
    biasp = np.einsum('hg,hnm->gnm', th1, bias_full) + th1_b[:, None, None]

    w9 = g['vl_w'].reshape(9, DH)                                  # [tap, c]
    w_eff = (w9 * g['vl_g'][None, :]).astype(f32)                  # [tap, c]
    sumw = np.zeros((DH, N), f32)
    for t in range(9):
        dy, dx = t // 3 - 1, t % 3 - 1
        for s in range(N):
            y, x = s // 7, s % 7
            if 0 <= y + dy < 7 and 0 <= x + dx < 7:
                sumw[:, s] += w9[t]
    s2 = th2.sum(axis=0) + N * th2_b                               # [g]
    B2 = (g['vl_g'][:, None] * (b_v[:, None] * sumw + g['vl_b'][:, None])
          + g['vl_beta'][:, None]
          + (b_v * s2[np.repeat(np.arange(H), D)])[:, None])       # [c, s=49]

    W_p = g['proj_w'] * g['proj_g'][None, :]
    b_p = g['proj_b'] * g['proj_g'] + g['proj_beta']

    consts = {}
    wqk = np.concatenate([W_q, W_k], axis=1).reshape(3, 128, 512)
    consts['wqk'] = wqk.astype(_BF16)
    consts['wv'] = W_v.reshape(3, 128, DH).astype(_BF16)
    consts['wp'] = W_p.reshape(8, 128, DIM).astype(_BF16)
    consts['bqk'] = np.concatenate([b_q, b_k]).reshape(4, 128).astype(f32)
    consts['bp'] = b_p.reshape(3, 128).astype(f32)

    # Talking heads as [jo, ji, K=128, M=128] block matrices in the
    # (hh, slot64) row space: row (hh*64 + slot(m)) of input tile ji =
    # head (2*ji+hh), key m; col likewise for output tile jo.
    def th_blocks(thw):
        Wb = np.zeros((4, 4, 128, 128), f32)
        eye = np.zeros((SL, SL), f32)
        eye[_SLOTS, _SLOTS] = 1.0
        for jo in range(4):
            for ji in range(4):
                for hhi in range(2):
                    for hho in range(2):
                        c = thw[2 * ji + hhi, 2 * jo + hho]
                        Wb[jo, ji, hhi * 64:hhi * 64 + SL,
                           hho * 64:hho * 64 + SL] += c * eye
        return Wb
    consts['w1s'] = th_blocks(th1).astype(_BF16)
    consts['w2s'] = th_blocks(th2).astype(_BF16)

    sel = np.zeros((128, 2), f32)
    sel[_SLOTS, 0] = 1.0
    sel[64 + _SLOTS, 1] = 1.0
    consts['sel'] = sel.astype(_BF16)

    dlt = np.zeros((128, 128), f32)
    for j in range(4):
        dlt[32 * j + 0, 0:64] = 1.0
        dlt[32 * j + 1, 64:128] = 1.0
    consts['dlt'] = dlt.astype(f32)

    # exp of the th1-transformed rel-pos bias in [(hh, slot) x (img, n)]
    # rows, replicated over 8 images (multiplied into E post-exp; guard
    # rows exp(0)=1).
    bsb = np.zeros((4, 128, GW), f32)
    for j in range(4):
        for hh in range(2):
            b = biasp[2 * j + hh].T                                # [m, n]
            bsb[j, hh * 64 + _SLOTS] = np.tile(b, (1, 8))
    consts['biasp'] = np.exp(bsb).astype(_BF16)

    # DVE tap weights: sbuf [128, 8, 9] (c-part, ct, tap)
    consts['w9t'] = w_eff.reshape(9, 8, 128).transpose(2, 1, 0).copy().astype(f32)

    # PE tap diagonal weights: [8 ct, n_pe, 128, 128]
    dw = np.zeros((8, len(TAPS_PE), 128, 128), f32)
    for ct in range(8):
        for ti, (dy, dx) in enumerate(TAPS_PE):
            tap = (dy + 1) * 3 + (dx + 1)
            np.fill_diagonal(dw[ct, ti], w_eff[tap, ct * 128:(ct + 1) * 128])
    consts['dw'] = dw.astype(_BF16)

    # B2 in padded-slot layout, replicated over the 8 images of a group:
    # [8, 128, 512] (zeros at guard slots)
    b2p = np.zeros((8, 128, SL), f32)
    b2p[:, :, _SLOTS] = B2.reshape(8, 128, N)
    consts['b2p'] = np.tile(b2p, (1, 1, 8)).astype(_BF16)

    consts['ident'] = np.eye(128, dtype=f32).astype(_BF16)
    return consts


def build_program(n_imgs, loop_n=1, stage=9):
    """Build the Bass program for one core processing n_imgs images.

    loop_n > 1 wraps the whole compute (including I/O DMA) in a hardware
    loop — used only by the timing harness to measure per-iteration HW time.
    """
    from contextlib import ExitStack
    import concourse.bass as bass
    import concourse.tile as tile
    from concourse import bacc, mybir

    f32 = mybir.dt.float32
    bf16 = mybir.dt.bfloat16
    AF = mybir.ActivationFunctionType
    ALU = mybir.AluOpType

    NI = n_imgs
    NG = NI // 8                 # groups of 8 images
    NT = NI * N                  # tokens

    nc = bacc.Bacc("TRN2", target_bir_lowering=False, debug=False,
                   enable_asserts=False)

    x_d = nc.dram_tensor("x", [3, 128, NT], bf16, kind="ExternalInput").ap()
    wqk_d = nc.dram_tensor("wqk", [3, 128, 512], bf16, kind="ExternalInput").ap()
    wv_d = nc.dram_tensor("wv", [3, 128, DH], bf16, kind="ExternalInput").ap()
    wp_d = nc.dram_tensor("wp", [8, 128, DIM], bf16, kind="ExternalInput").ap()
    bqk_d = nc.dram_tensor("bqk", [4, 128], f32, kind="ExternalInput").ap()
    bp_d = nc.dram_tensor("bp", [3, 128], f32, kind="ExternalInput").ap()
    w1_d = nc.dram_tensor("w1s", [4, 4, 128, 128], bf16, kind="ExternalInput").ap()
    w2_d = nc.dram_tensor("w2s", [4, 4, 128, 128], bf16, kind="ExternalInput").ap()
    sel_d = nc.dram_tensor("sel", [128, 2], bf16, kind="ExternalInput").ap()
    dlt_d = nc.dram_tensor("dlt", [128, 128], f32, kind="ExternalInput").ap()
    bias_d = nc.dram_tensor("biasp", [4, 128, GW], bf16, kind="ExternalInput").ap()
    w9_d = nc.dram_tensor("w9t", [128, 8, 9], f32, kind="ExternalInput").ap()
    dw_d = nc.dram_tensor("dw", [8, len(TAPS_PE), 128, 128], bf16,
                          kind="ExternalInput").ap()
    b2_d = nc.dram_tensor("b2p", [8, 128, GWP], bf16, kind="ExternalInput").ap()
    id_d = nc.dram_tensor("ident", [128, 128], bf16, kind="ExternalInput").ap()
    out_d = nc.dram_tensor("out", [3, 128, NT], bf16, kind="ExternalOutput").ap()

    with tile.TileContext(nc) as tc, ExitStack() as ctx:
        const = ctx.enter_context(tc.tile_pool(name="const", bufs=1))
        pers = ctx.enter_context(tc.tile_pool(name="pers", bufs=1))
        mid = ctx.enter_context(tc.tile_pool(name="mid", bufs=6))
        accp = ctx.enter_context(tc.tile_pool(name="accp", bufs=2))
        stg = ctx.enter_context(tc.tile_pool(name="stg", bufs=3))
        ps = ctx.enter_context(tc.tile_pool(name="ps", bufs=8, space="PSUM"))

        dma = nc.sync.dma_start

        # ---------------- constants ----------------
        # issue order = scheduler priority: small / first-needed tiles first,
        # the large talking-heads + conv-diag tables last
        bqk_t = const.tile([128, 4], f32, name="bqk", tag="bqk")
        dma(out=bqk_t, in_=bass.AP(tensor=bqk_d.tensor, offset=0,
                                   ap=[[1, 128], [128, 4]]))
        bp_t = const.tile([128, 3], f32, name="bp", tag="bp")
        dma(out=bp_t, in_=bass.AP(tensor=bp_d.tensor, offset=0,
                                  ap=[[1, 128], [128, 3]]))
        sel_t = const.tile([128, 2], bf16, name="sel", tag="sel")
        dma(out=sel_t, in_=sel_d)
        dlt_t = const.tile([128, 128], f32, name="dlt", tag="dlt")
        dma(out=dlt_t, in_=dlt_d)
        w9_t = const.tile([128, 8, 9], f32, name="w9", tag="w9")
        dma(out=w9_t, in_=w9_d)
        id_t = const.tile([128, 128], bf16, name="id", tag="id")
        dma(out=id_t, in_=id_d)
        wqk_t = [const.tile([128, 512], bf16, name=f"wqk{k}", tag=f"wqk{k}") for k in range(3)]
        wv_t = [const.tile([128, DH], bf16, name=f"wv{k}", tag=f"wv{k}") for k in range(3)]
        wp_t = [const.tile([128, DIM], bf16, name=f"wp{k}", tag=f"wp{k}") for k in range(8)]
        for k in range(3):
            dma(out=wqk_t[k], in_=wqk_d[k])
            dma(out=wv_t[k], in_=wv_d[k])
        bias_t = [const.tile([128, GW], bf16, name=f"bi{j}", tag=f"bi{j}") for j in range(4)]
        for j in range(4):
            dma(out=bias_t[j], in_=bias_d[j])
        b2c = [const.tile([128, GWP], bf16, name=f"b2c{c}", tag=f"b2c{c}")
               for c in range(8)]
        for c in range(8):
            dma(out=b2c[c], in_=b2_d[c])
        for k in range(8):
            dma(out=wp_t[k], in_=wp_d[k])
        w1_t = const.tile([128, 16, 128], bf16, name="w1", tag="w1")
        dma(out=w1_t, in_=bass.AP(tensor=w1_d.tensor, offset=0,
                                  ap=[[128, 128], [128 * 128, 16], [1, 128]]))
        w2_t = const.tile([128, 16, 128], bf16, name="w2", tag="w2")
        dma(out=w2_t, in_=bass.AP(tensor=w2_d.tensor, offset=0,
                                  ap=[[128, 128], [128 * 128, 16], [1, 128]]))
        dw_t = const.tile([128, 8 * len(TAPS_PE), 128], bf16, name="dw", tag="dw")
        dma(out=dw_t, in_=bass.AP(tensor=dw_d.tensor, offset=0,
                                  ap=[[128, 128], [128 * 128, 8 * len(TAPS_PE)],
                                      [1, 128]]))

        # ---------------- persistent / slot tiles ----------------
        xg = [[pers.tile([128, GW], bf16, name=f"xg{k}_{s}", tag=f"xg{k}_{s}")
               for s in range(2)] for k in range(3)]
        qcm = [[pers.tile([128, GW], bf16, name=f"q{t}_{s}", tag=f"q{t}_{s}")
                for s in range(2)] for t in range(2)]
        kcm = [[pers.tile([128, GWP], bf16, name=f"k{t}_{s}", tag=f"k{t}_{s}")
                for s in range(2)] for t in range(2)]
        vcm = [[pers.tile([128, GWP + 2 * PADG], bf16, name=f"vc{c}_{s}",
                          tag=f"vc{c}_{s}")
                for s in range(2)] for c in range(8)]
        vtokE = [pers.tile([64, DH], bf16, name=f"vtE{s}", tag=f"vtE{s}") for s in range(8)]
        vtokO = [pers.tile([64, DH], bf16, name=f"vtO{s}", tag=f"vtO{s}") for s in range(8)]
        Ls = [[pers.tile([128, GW], bf16, name=f"Ls{j}_{s}", tag=f"Ls{j}_{s}") for s in range(2)]
              for j in range(4)]
        a2h = [[[pers.tile([64, GW], bf16, name=f"a2_{j}_{hh}_{s}",
                            tag=f"a2_{j}_{hh}_{s}") for s in range(2)]
                 for hh in range(2)] for j in range(4)]
        r_sb = [pers.tile([128, GW], f32, name=f"rsb{s}", tag=f"rsb{s}") for s in range(2)]

        # one-time zero init: padded tiles fully (guard slots must stay 0)
        for c in range(8):
            for s in range(2):
                nc.vector.memset(vcm[c][s], 0.0)
        for t in range(2):
            for s in range(2):
                nc.vector.memset(kcm[t][s], 0.0)

        # 4-d views of a padded group region: [p, i, y(7), x(7)] valid slots
        def padview(tile_, base):
            v = tile_[:, base:base + GWP].rearrange("p (i q) -> p i q", q=SL)
            v = v.rearrange("p i (y x) -> p i y x", x=8)
            return v[:, :, 0:7, 0:7]

        def cview(tile_):
            return tile_.rearrange("p (i y x) -> p i y x", y=7, x=7)

        def group_body(g):
            sl = g % 2          # phase slot
            c0 = g * GW

            # --- x load (channel-major direct) ---
            for kt in range(3):
                dma(out=xg[kt][sl], in_=x_d[kt][:, c0:c0 + GW])

            # --- QKV channel-major ---
            for mt in range(12):
                qp = ps.tile([128, 512], f32, name="ps", tag="ps")
                for kt in range(3):
                    if mt < 4:
                        w = wqk_t[kt][:, mt * 128:(mt + 1) * 128]
                    else:
                        w = wv_t[kt][:, (mt - 4) * 128:(mt - 3) * 128]
                    nc.tensor.matmul(qp[:, 0:GW], w,
                                     xg[kt][sl],
                                     start=(kt == 0), stop=(kt == 2))
                if mt < 2:
                    nc.scalar.activation(qcm[mt][sl], qp[:, 0:GW],
                                         AF.Identity,
                                         bias=bqk_t[:, mt:mt + 1])
                elif mt < 4:
                    # k with bias, scattered into the padded-slot layout
                    sc = mid.tile([128, GW], bf16, name="ksc", tag="ksc")
                    nc.scalar.activation(sc, qp[:, 0:GW], AF.Identity,
                                         bias=bqk_t[:, mt:mt + 1])
                    nc.vector.tensor_copy(padview(kcm[mt - 2][sl], 0), cview(sc))
                elif mt % 2 == 0:
                    nc.vector.tensor_copy(padview(vcm[mt - 4][sl], PADG),
                                          cview(qp[:, 0:GW]))
                else:
                    nc.scalar.activation(padview(vcm[mt - 4][sl], PADG),
                                         cview(qp[:, 0:GW]), AF.Copy)

            # --- v token-major via PE pair-transposes ---
            if stage < 2:
                return
            for pr in range(4):
                p = 4 * g + pr
                vp = [ps.tile([128, 512], f32, name="ps", tag="ps") for _ in range(2)]
                for ct in range(8):
                    nh, cc = ct // 4, (ct % 4) * 128
                    nc.tensor.matmul(
                        vp[nh][:, cc:cc + 128],
                        vcm[ct][sl][:, PADG + pr * 128: PADG + (pr + 1) * 128],
                        id_t, start=True, stop=True)
                nc.vector.tensor_copy(vtokE[p % 8][:, 0:512], vp[0][0:64, :])
                nc.scalar.activation(vtokO[p % 8][:, 0:512],
                                     vp[0][64:128, :], AF.Copy)
                nc.vector.tensor_copy(vtokE[p % 8][:, 512:1024], vp[1][0:64, :])
                nc.scalar.activation(vtokO[p % 8][:, 512:1024],
                                     vp[1][64:128, :], AF.Copy)

            # --- depthwise conv: DVE taps into B2-seeded acc ---
            if stage < 3:
                return
            acc_t = []
            for ct in range(8):
                acc = accp.tile([128, GWP], bf16, name=f"acc{ct}", tag=f"acc{ct}")
                accv3 = acc.rearrange("p (k x) -> p k x", x=8)
                b2v3 = b2c[ct].rearrange("p (k x) -> p k x", x=8)
                srcb = vcm[ct][sl]
                for dy, dx in TAPS_DVE:
                    tap = (dy + 1) * 3 + (dx + 1)
                    dlta = 8 * dy + dx
                    xs_o = slice(max(0, -dx), 7 - max(0, dx))
                    xs_i = slice(max(0, dx) + PADG % 8, 7 - max(0, -dx) + PADG % 8)
                    src3 = bass.AP(tensor=srcb.tensor,
                                   offset=srcb.offset + PADG + 8 * dy,
                                   ap=[[srcb.ap[0][0], 128], [8, 64], [1, 8]])
                    first = (dy, dx) == TAPS_DVE[0]
                    if first:
                        # seed guard slots too (assembly never reads them,
                        # but keep the accumulate chain well-defined)
                        nc.vector.scalar_tensor_tensor(
                            out=acc, in0=srcb[:, PADG:PADG + GWP],
                            scalar=w9_t[:, ct, tap:tap + 1],
                            in1=b2c[ct], op0=ALU.mult, op1=ALU.add)
                    else:
                        nc.vector.scalar_tensor_tensor(
                            out=accv3[:, :, xs_o],
                            in0=src3[:, :, xs_o.start + dx: xs_o.stop + dx],
                            scalar=w9_t[:, ct, tap:tap + 1],
                            in1=accv3[:, :, xs_o], op0=ALU.mult, op1=ALU.add)
                acc_t.append(acc)

            # --- qk logits (rows = (hh, slot64)) ---
            if stage < 4:
                return
            Lp = [ps.tile([128, 512], f32, name="ps", tag="ps") for _ in range(4)]
            for ig in range(8):
                for h in range(H):
                    j, hh = h // 2, h % 2
                    t4, row = h // 4, (h % 4) * 32
                    nc.tensor.matmul(
                        Lp[j][64 * hh: 64 * hh + SL, ig * N:(ig + 1) * N],
                        kcm[t4][sl][row:row + 32, ig * SL:(ig + 1) * SL],
                        qcm[t4][sl][row:row + 32, ig * N:(ig + 1) * N],
                        start=True, stop=True,
                        tile_position=(row, 64 * hh))
            for j in range(4):
                with tc.high_priority(700):
                    nc.scalar.activation(Ls[j][sl], Lp[j][:, 0:GW], AF.Copy)

            # --- talking heads 1 (+ rel-pos bias) + exp ---
            if stage < 5:
                return
            E = []
            L2p = [ps.tile([128, 512], f32, name="ps", tag="ps") for _ in range(4)]
            for jo in range(4):
                for ji in range(4):
                    nc.tensor.matmul(L2p[jo][:, 0:GW],
                                     w1_t[:, jo * 4 + ji, :],
                                     Ls[ji][sl],
                                     start=(ji == 0), stop=(ji == 3))
            for jo in range(4):
                e0 = mid.tile([128, GW], bf16, name="E0", tag="E0", bufs=4)
                e = mid.tile([128, GW], bf16, name="E", tag="E")
                with tc.high_priority(700):
                    nc.scalar.activation(e0, L2p[jo][:, 0:GW], AF.Exp)
                    nc.vector.tensor_mul(e, e0, bias_t[jo])
                E.append(e)

            # --- softmax denominator ---
            if stage < 6:
                return
            csp = ps.tile([128, 512], f32, name="ps", tag="ps")
            for j in range(4):
                nc.tensor.matmul(csp[32 * j: 32 * j + 2, 0:GW], sel_t, E[j],
                                 start=True, stop=True,
                                 tile_position=(0, 32 * j))
            with tc.high_priority(700):
                for j in range(4):
                    nc.vector.reciprocal(r_sb[sl][32 * j: 32 * j + 2, :],
                                         csp[32 * j: 32 * j + 2, 0:GW])

            # --- normalize + talking heads 2 ---
            A = []
            for j in range(4):
                rp = ps.tile([128, 512], f32, name="ps", tag="ps")
                nc.tensor.matmul(rp[:, 0:GW], dlt_t[32 * j: 32 * j + 2, :],
                                 r_sb[sl][32 * j: 32 * j + 2, :],
                                 start=True, stop=True,
                                 tile_position=(32 * j, 0))
                a = mid.tile([128, GW], bf16, name="A", tag="A")
                with tc.high_priority(700):
                    nc.vector.tensor_mul(a, E[j], rp[:, 0:GW])
                A.append(a)
            A2p = [ps.tile([128, 512], f32, name="ps", tag="ps") for _ in range(4)]
            for jo in range(4):
                for ji in range(4):
                    nc.tensor.matmul(A2p[jo][:, 0:GW],
                                     w2_t[:, jo * 4 + ji, :],
                                     A[ji],
                                     start=(ji == 0), stop=(ji == 3))
            for jo in range(4):
                with tc.high_priority(700):
                    nc.scalar.activation(a2h[jo][0][sl], A2p[jo][0:64, 0:GW],
                                         AF.Copy)
                    nc.scalar.activation(a2h[jo][1][sl], A2p[jo][64:128, 0:GW],
                                         AF.Copy)

            # --- attention * V (+ PE conv taps), assembly, relu ---
            if stage < 7:
                return
            relu_t = []
            for ct in range(8):
                op2 = ps.tile([128, 512], f32, name="ps", tag="ps")
                jo, hh = ct // 2, ct % 2
                for ig in range(8):
                    i = 8 * g + ig
                    pp = ig % 2
                    vt = (vtokE if pp == 0 else vtokO)[(i // 2) % 8]
                    nc.tensor.matmul(
                        op2[:, ig * N:(ig + 1) * N],
                        vt[0:SL, ct * 128:(ct + 1) * 128],
                        a2h[jo][hh][sl][0:SL, ig * N:(ig + 1) * N],
                        start=True, stop=True)
                if stage >= 8:
                    cps = ps.tile([128, 512], f32, name="ps", tag="ps")
                    for ti, (dy, dx) in enumerate(TAPS_PE):
                        dlta = 8 * dy + dx
                        nc.tensor.matmul(
                            cps[:, 0:GWP],
                            dw_t[:, ct * len(TAPS_PE) + ti, :],
                            vcm[ct][sl][:, PADG + dlta: PADG + dlta + GWP],
                            start=(ti == 0), stop=(ti == len(TAPS_PE) - 1))
                tmp = mid.tile([128, GW], bf16, name="tmp", tag="tmp", bufs=3)
                op2d = mid.tile([128, GW], bf16, name="op2d", tag="op2d", bufs=3)
                opv = op2[:, 0:GW].rearrange("p (i y x) -> p i y x", y=7, x=7)
                nc.scalar.activation(op2d, op2[:, 0:GW], AF.Copy)
                accv = acc_t[ct].rearrange("p (i q) -> p i q", q=SL)
                accv = accv.rearrange("p i (y x) -> p i y x",
                                      x=8)[:, :, 0:7, 0:7]
                nc.vector.tensor_add(cview(tmp), cview(op2d), accv)
                if stage >= 8:
                    cpsd = mid.tile([128, GW], bf16, name="cpsd", tag="cpsd",
                                    bufs=3)
                    cpsv = cps[:, 0:GWP].rearrange("p (i q) -> p i q", q=SL)
                    cpsv = cpsv.rearrange("p i (y x) -> p i y x", x=8)[:, :, 0:7, 0:7]
                    nc.scalar.activation(cview(cpsd), cpsv, AF.Copy)
                    nc.vector.tensor_add(cview(tmp), cview(tmp), cview(cpsd))
                rl = mid.tile([128, GW], bf16, name="rl", tag="rl", bufs=10)
                nc.vector.tensor_scalar_max(rl, tmp, 0.0)
                relu_t.append(rl)

            # --- projection + store ---
            for mt in range(3):
                st = stg.tile([128, GW], bf16, name="st", tag="st")
                pp_ = ps.tile([128, 512], f32, name="ps", tag="ps")
                for kt in range(8):
                    nc.tensor.matmul(pp_[:, 0:GW],
                                     wp_t[kt][:, mt * 128:(mt + 1) * 128],
                                     relu_t[kt],
                                     start=(kt == 0), stop=(kt == 7))
                nc.scalar.activation(st, pp_[:, 0:GW], AF.Identity,
                                     bias=bp_t[:, mt:mt + 1])
                dma(out=out_d[mt][:, c0:c0 + GW], in_=st)

        if loop_n > 1:
            with tc.For_i(0, loop_n, 1):
                for g in range(NG):
                    group_body(g)
        else:
            for g in range(NG):
                group_body(g)

    nc.compile()
    return nc


_CACHE = {}


def _get_program(n_imgs):
    if n_imgs not in _CACHE:
        _CACHE[n_imgs] = build_program(n_imgs)
    return _CACHE[n_imgs]


def make_in_maps(inputs, n_cores=NCORES):
    """Host prep: shard + channel-major x, build replicated constants."""
    consts = make_consts(inputs)
    x = np.asarray(inputs['x'], np.float32)
    B = x.shape[0]
    ni = B // n_cores
    nt = ni * N
    x = x.reshape(B, N, DIM)
    in_maps = []
    for c in range(n_cores):
        m = dict(consts)
        xc = x[c * ni:(c + 1) * ni].reshape(nt, DIM).T    # [384, nt]
        m['x'] = np.ascontiguousarray(xc).reshape(3, 128, nt).astype(_BF16)
        in_maps.append(m)
    return in_maps, ni


def assemble_out(results, ni):
    """[3,128,nt] bf16 per core -> full [B, R, R, DIM] f32."""
    nt = ni * N
    outs = []
    for r in results:
        oc = np.asarray(r['out'], np.float32).reshape(DIM, nt)
        outs.append(oc.T.reshape(ni, R, R, DIM))
    return np.concatenate(outs, axis=0)


def kernel(**inputs):
    from concourse import bass_utils
    in_maps, ni = make_in_maps(inputs)
    nc = _get_program(ni)
    res = bass_utils.run_bass_kernel_spmd(
        nc, in_maps, core_ids=list(range(NCORES)))
    return assemble_out(res.results, ni).astype(np.float32)


# revision 50
# speedup vs baseline: 4800.2375x; 2302.6526x over previous
"""Trainium2 Bass kernel for nn_Attention4D_77644418777285.

Attention4D block (EfficientViT-style): 1x1-conv QKV + BN, depthwise-3x3
local-V branch, relative-position bias, talking-heads attention (8 heads,
49 tokens), projection. Batch 512 sharded 64-per-core across 8 NeuronCores
(pure data parallel; weights replicated).

v3 layout (per core, 64 images, groups of 8 images):
  - x arrives channel-major from host ([3,128,NT] bf16); output returned
    channel-major bf16 and re-transposed on host — no PE transposes.
  - Key-token axis (m) padded to 64 slots per image (8x8 grid, shared
    guard row/col, one-time zeroed): every shifted view is a plain 2-D
    column offset, v token-major tiles come from a single [128,128]
    PE transpose-matmul per (image-pair, channel-tile), and the attention
    middle (logits rows, talking-head blocks, selector, bias) runs in the
    64-slot row space with zero weights on guard slots.
  - depthwise 3x3 conv split between DVE (scalar_tensor_tensor taps into
    a B2-seeded SBUF accumulator) and PE (per-channel diagonal-matrix
    matmuls accumulating shifted windows in PSUM).
  - talking-heads + rel-pos bias injected pre-exp into the th1 psum chain;
    softmax denominators via selector matmul, batched DVE reciprocal,
    normalization broadcast via constant delta matmul.
  - assembly o + v_local(+psum part) + relu, projection, straight DMA out.
"""

import numpy as np
import ml_dtypes

R = 7
N = 49
H = 8
KD = 32
D = 128
DH = 1024
DIM = 384
SCALE = KD ** -0.5
NCORES = 8
B_FULL = 512
GW = 8 * N          # 392 compact cols per group of 8 images
SL = 64             # padded slots per image (8x8)
GWP = 8 * SL        # 512 padded cols per group
PADG = 16           # guard cols at each end of padded tiles

# conv tap split: (dy, dx) lists
TAPS_DVE = [(0, 0), (0, -1), (0, 1)]
TAPS_PE = [(-1, -1), (-1, 0), (-1, 1), (1, -1), (1, 0), (1, 1)]

_BF16 = ml_dtypes.bfloat16


def _bias_idxs(r):
    pos = np.stack(np.meshgrid(np.arange(r), np.arange(r))).reshape(2, -1)
    rel = np.abs(pos[:, :, None] - pos[:, None, :])
    return (rel[0] * r + rel[1]).reshape(-1)


def _slot(t):
    """compact token t (0..48) -> padded slot (0..63)."""
    return (t // 7) * 8 + (t % 7)


_SLOTS = np.array([_slot(t) for t in range(N)])


def make_consts(inp):
    f32 = np.float32
    g = {k: np.asarray(v, f32) for k, v in inp.items()}

    th1, th1_b = g['th1_w'], g['th1_b']
    th2, th2_b = g['th2_w'], g['th2_b']

    W_q = g['q_w'] * g['q_g'][None, :] * SCALE
    b_q = (g['q_b'] * g['q_g'] + g['q_beta']) * SCALE
    W_k = g['k_w'] * g['k_g'][None, :]
    b_k = g['k_b'] * g['k_g'] + g['k_beta']
    W_v = g['v_w'] * g['v_g'][None, :]
    b_v = g['v_b'] * g['v_g'] + g['v_beta']

    idxs = _bias_idxs(R)
    bias_full = g['attn_bias'][:, idxs].reshape(H, N, N)          # [h, n, m]
    biasp = np.einsum('hg,hnm->gnm', th1, bias_full) + th1_b[:, None, None]

    w9 = g['vl_w'].reshape(9, DH)                                  # [tap, c]
    w_eff = (w9 * g['vl_g'][None, :]).astype(f32)                  # [tap, c]
    sumw = np.zeros((DH, N), f32)
    for t in range(9):
        dy, dx = t // 3 - 1, t % 3 - 1
        for s in range(N):
            y, x = s // 7, s % 7
            if 0 <= y + dy < 7 and 0 <= x + dx < 7:
                sumw[:, s] += w9[t]
    s2 = th2.sum(axis=0) + N * th2_b                               # [g]
    B2 = (g['vl_g'][:, None] * (b_v[:, None] * sumw + g['vl_b'][:, None])
          + g['vl_beta'][:, None]
          + (b_v * s2[np.repeat(np.arange(H), D)])[:, None])       # [c, s=49]

    W_p = g['proj_w'] * g['proj_g'][None, :]
    b_p = g['proj_b'] * g['proj_g'] + g['proj_beta']

    consts = {}
    wqk = np.concatenate([W_q, W_k], axis=1).reshape(3, 128, 512)
    consts['wqk'] = wqk.astype(_BF16)
    consts['wv'] = W_v.reshape(3, 128, DH).astype(_BF16)
    consts['wp'] = W_p.reshape(8, 128, DIM).astype(_BF16)
    consts['bqk'] = np.concatenate([b_q, b_k]).reshape(4, 128).astype(f32)
    consts['bp'] = b_p.reshape(3, 128).astype(f32)

    # Talking heads as [jo, ji, K=128, M=128] block matrices in the
    # (hh, slot64) row space: row (hh*64 + slot(m)) of input tile ji =
    # head (2*ji+hh), key m; col likewise for output tile jo.
    def th_blocks(thw):
        Wb = np.zeros((4, 4, 128, 128), f32)
        eye = np.zeros((SL, SL), f32)
        eye[_SLOTS, _SLOTS] = 1.0
        for jo in range(4):
            for ji in range(4):
                for hhi in range(2):
                    for hho in range(2):
                        c = thw[2 * ji + hhi, 2 * jo + hho]
                        Wb[jo, ji, hhi * 64:hhi * 64 + SL,
                           hho * 64:hho * 64 + SL] += c * eye
        return Wb
    consts['w1s'] = th_blocks(th1).astype(_BF16)
    consts['w2s'] = th_blocks(th2).astype(_BF16)

    sel = np.zeros((128, 2), f32)
    sel[_SLOTS, 0] = 1.0
    sel[64 + _SLOTS, 1] = 1.0
    consts['sel'] = sel.astype(_BF16)

    dlt = np.zeros((128, 128), f32)
    for j in range(4):
        dlt[32 * j + 0, 0:64] = 1.0
        dlt[32 * j + 1, 64:128] = 1.0
    consts['dlt'] = dlt.astype(f32)

    # exp of the th1-transformed rel-pos bias in [(hh, slot) x (img, n)]
    # rows, replicated over 8 images (multiplied into E post-exp; guard
    # rows exp(0)=1).
    bsb = np.zeros((4, 128, GW), f32)
    for j in range(4):
        for hh in range(2):
            b = biasp[2 * j + hh].T                                # [m, n]
            bsb[j, hh * 64 + _SLOTS] = np.tile(b, (1, 8))
    consts['biasp'] = np.exp(bsb).astype(_BF16)

    # DVE tap weights: sbuf [128, 8, 9] (c-part, ct, tap)
    consts['w9t'] = w_eff.reshape(9, 8, 128).transpose(2, 1, 0).copy().astype(f32)

    # PE tap diagonal weights: [8 ct, n_pe, 128, 128]
    dw = np.zeros((8, len(TAPS_PE), 128, 128), f32)
    for ct in range(8):
        for ti, (dy, dx) in enumerate(TAPS_PE):
            tap = (dy + 1) * 3 + (dx + 1)
            np.fill_diagonal(dw[ct, ti], w_eff[tap, ct * 128:(ct + 1) * 128])
    consts['dw'] = dw.astype(_BF16)

    # B2 in padded-slot layout, replicated over the 8 images of a group:
    # [8, 128, 512] (zeros at guard slots)
    b2p = np.zeros((8, 128, SL), f32)
    b2p[:, :, _SLOTS] = B2.reshape(8, 128, N)
    consts['b2p'] = np.tile(b2p, (1, 1, 8)).astype(_BF16)

    consts['ident'] = np.eye(128, dtype=f32).astype(_BF16)
    return consts


def build_program(n_imgs, loop_n=1, stage=9):
    """Build the Bass program for one core processing n_imgs images.

    loop_n > 1 wraps the whole compute (including I/O DMA) in a hardware
    loop — used only by the timing harness to measure per-iteration HW time.
    """
    from contextlib import ExitStack
    import concourse.bass as bass
    import concourse.tile as tile
    from concourse import bacc, mybir

    f32 = mybir.dt.float32
    bf16 = mybir.dt.bfloat16
    AF = mybir.ActivationFunctionType
    ALU = mybir.AluOpType

    NI = n_imgs
    NG = NI // 8                 # groups of 8 images
    NT = NI * N                  # tokens

    nc = bacc.Bacc("TRN2", target_bir_lowering=False, debug=False,
                   enable_asserts=False)

    x_d = nc.dram_tensor("x", [3, 128, NT], bf16, kind="ExternalInput").ap()
    wqk_d = nc.dram_tensor("wqk", [3, 128, 512], bf16, kind="ExternalInput").ap()
    wv_d = nc.dram_tensor("wv", [3, 128, DH], bf16, kind="ExternalInput").ap()
    wp_d = nc.dram_tensor("wp", [8, 128, DIM], bf16, kind="ExternalInput").ap()
    bqk_d = nc.dram_tensor("bqk", [4, 128], f32, kind="ExternalInput").ap()
    bp_d = nc.dram_tensor("bp", [3, 128], f32, kind="ExternalInput").ap()
    w1_d = nc.dram_tensor("w1s", [4, 4, 128, 128], bf16, kind="ExternalInput").ap()
    w2_d = nc.dram_tensor("w2s", [4, 4, 128, 128], bf16, kind="ExternalInput").ap()
    sel_d = nc.dram_tensor("sel", [128, 2], bf16, kind="ExternalInput").ap()
    dlt_d = nc.dram_tensor("dlt", [128, 128], f32, kind="ExternalInput").ap()
    bias_d = nc.dram_tensor("biasp", [4, 128, GW], bf16, kind="ExternalInput").ap()
    w9_d = nc.dram_tensor("w9t", [128, 8, 9], f32, kind="ExternalInput").ap()
    dw_d = nc.dram_tensor("dw", [8, len(TAPS_PE), 128, 128], bf16,
                          kind="ExternalInput").ap()
    b2_d = nc.dram_tensor("b2p", [8, 128, GWP], bf16, kind="ExternalInput").ap()
    id_d = nc.dram_tensor("ident", [128, 128], bf16, kind="ExternalInput").ap()
    out_d = nc.dram_tensor("out", [3, 128, NT], bf16, kind="ExternalOutput").ap()

    with tile.TileContext(nc) as tc, ExitStack() as ctx:
        const = ctx.enter_context(tc.tile_pool(name="const", bufs=1))
        pers = ctx.enter_context(tc.tile_pool(name="pers", bufs=1))
        mid = ctx.enter_context(tc.tile_pool(name="mid", bufs=6))
        accp = ctx.enter_context(tc.tile_pool(name="accp", bufs=2))
        stg = ctx.enter_context(tc.tile_pool(name="stg", bufs=3))
        ps = ctx.enter_context(tc.tile_pool(name="ps", bufs=8, space="PSUM"))

        dma = nc.sync.dma_start

        # ---------------- constants ----------------
        # issue order = scheduler priority: small / first-needed tiles first,
        # the large talking-heads + conv-diag tables last
        bqk_t = const.tile([128, 4], f32, name="bqk", tag="bqk")
        dma(out=bqk_t, in_=bass.AP(tensor=bqk_d.tensor, offset=0,
                                   ap=[[1, 128], [128, 4]]))
        bp_t = const.tile([128, 3], f32, name="bp", tag="bp")
        dma(out=bp_t, in_=bass.AP(tensor=bp_d.tensor, offset=0,
                                  ap=[[1, 128], [128, 3]]))
        sel_t = const.tile([128, 2], bf16, name="sel", tag="sel")
        dma(out=sel_t, in_=sel_d)
        dlt_t = const.tile([128, 128], f32, name="dlt", tag="dlt")
        dma(out=dlt_t, in_=dlt_d)
        w9_t = const.tile([128, 8, 9], f32, name="w9", tag="w9")
        dma(out=w9_t, in_=w9_d)
        id_t = const.tile([128, 128], bf16, name="id", tag="id")
        dma(out=id_t, in_=id_d)
        wqk_t = [const.tile([128, 512], bf16, name=f"wqk{k}", tag=f"wqk{k}") for k in range(3)]
        wv_t = [const.tile([128, DH], bf16, name=f"wv{k}", tag=f"wv{k}") for k in range(3)]
        wp_t = [const.tile([128, DIM], bf16, name=f"wp{k}", tag=f"wp{k}") for k in range(8)]
        for k in range(3):
            dma(out=wqk_t[k], in_=wqk_d[k])
            dma(out=wv_t[k], in_=wv_d[k])
        bias_t = [const.tile([128, GW], bf16, name=f"bi{j}", tag=f"bi{j}") for j in range(4)]
        for j in range(4):
            dma(out=bias_t[j], in_=bias_d[j])
        b2c = [const.tile([128, GWP], bf16, name=f"b2c{c}", tag=f"b2c{c}")
               for c in range(8)]
        for c in range(8):
            dma(out=b2c[c], in_=b2_d[c])
        for k in range(8):
            dma(out=wp_t[k], in_=wp_d[k])
        w1_t = const.tile([128, 16, 128], bf16, name="w1", tag="w1")
        dma(out=w1_t, in_=bass.AP(tensor=w1_d.tensor, offset=0,
                                  ap=[[128, 128], [128 * 128, 16], [1, 128]]))
        w2_t = const.tile([128, 16, 128], bf16, name="w2", tag="w2")
        dma(out=w2_t, in_=bass.AP(tensor=w2_d.tensor, offset=0,
                                  ap=[[128, 128], [128 * 128, 16], [1, 128]]))
        dw_t = const.tile([128, 8 * len(TAPS_PE), 128], bf16, name="dw", tag="dw")
        dma(out=dw_t, in_=bass.AP(tensor=dw_d.tensor, offset=0,
                                  ap=[[128, 128], [128 * 128, 8 * len(TAPS_PE)],
                                      [1, 128]]))

        # ---------------- persistent / slot tiles ----------------
        xg = [[pers.tile([128, GW], bf16, name=f"xg{k}_{s}", tag=f"xg{k}_{s}")
               for s in range(2)] for k in range(3)]
        qcm = [[pers.tile([128, GW], bf16, name=f"q{t}_{s}", tag=f"q{t}_{s}")
                for s in range(2)] for t in range(2)]
        kcm = [[pers.tile([128, GWP], bf16, name=f"k{t}_{s}", tag=f"k{t}_{s}")
                for s in range(2)] for t in range(2)]
        vcm = [[pers.tile([128, GWP + 2 * PADG], bf16, name=f"vc{c}_{s}",
                          tag=f"vc{c}_{s}")
                for s in range(2)] for c in range(8)]
        vtokE = [pers.tile([64, DH], bf16, name=f"vtE{s}", tag=f"vtE{s}") for s in range(8)]
        vtokO = [pers.tile([64, DH], bf16, name=f"vtO{s}", tag=f"vtO{s}") for s in range(8)]
        Ls = [[pers.tile([128, GW], bf16, name=f"Ls{j}_{s}", tag=f"Ls{j}_{s}") for s in range(2)]
              for j in range(4)]
        a2h = [[[pers.tile([64, GW], bf16, name=f"a2_{j}_{hh}_{s}",
                            tag=f"a2_{j}_{hh}_{s}") for s in range(2)]
                 for hh in range(2)] for j in range(4)]
        r_sb = [pers.tile([128, GW], f32, name=f"rsb{s}", tag=f"rsb{s}") for s in range(2)]

        # one-time zero init: padded tiles fully (guard slots must stay 0)
        for c in range(8):
            for s in range(2):
                nc.vector.memset(vcm[c][s], 0.0)
        for t in range(2):
            for s in range(2):
                nc.vector.memset(kcm[t][s], 0.0)

        # 4-d views of a padded group region: [p, i, y(7), x(7)] valid slots
        def padview(tile_, base):
            v = tile_[:, base:base + GWP].rearrange("p (i q) -> p i q", q=SL)
            v = v.rearrange("p i (y x) -> p i y x", x=8)
            return v[:, :, 0:7, 0:7]

        def cview(tile_):
            return tile_.rearrange("p (i y x) -> p i y x", y=7, x=7)

        def group_body(g):
            sl = g % 2          # phase slot
            c0 = g * GW

            # --- x load (channel-major direct) ---
            for kt in range(3):
                dma(out=xg[kt][sl], in_=x_d[kt][:, c0:c0 + GW])

            # --- QKV channel-major ---
            for mt in range(12):
                qp = ps.tile([128, 512], f32, name="ps", tag="ps")
                for kt in range(3):
                    if mt < 4:
                        w = wqk_t[kt][:, mt * 128:(mt + 1) * 128]
                    else:
                        w = wv_t[kt][:, (mt - 4) * 128:(mt - 3) * 128]
                    nc.tensor.matmul(qp[:, 0:GW], w,
                                     xg[kt][sl],
                                     start=(kt == 0), stop=(kt == 2))
                if mt < 2:
                    nc.scalar.activation(qcm[mt][sl], qp[:, 0:GW],
                                         AF.Identity,
                                         bias=bqk_t[:, mt:mt + 1])
                elif mt < 4:
                    # k with bias, scattered into the padded-slot layout
                    sc = mid.tile([128, GW], bf16, name="ksc", tag="ksc")
                    nc.scalar.activation(sc, qp[:, 0:GW], AF.Identity,
                                         bias=bqk_t[:, mt:mt + 1])
                    nc.vector.tensor_copy(padview(kcm[mt - 2][sl], 0), cview(sc))
                elif mt % 2 == 0:
                    nc.vector.tensor_copy(padview(vcm[mt - 4][sl], PADG),
                                          cview(qp[:, 0:GW]))
                else:
                    nc.scalar.activation(padview(vcm[mt - 4][sl], PADG),
                                         cview(qp[:, 0:GW]), AF.Copy)

            # --- v token-major via PE pair-transposes ---
            if stage < 2:
                return
            for pr in range(4):
                p = 4 * g + pr
                vp = [ps.tile([128, 512], f32, name="ps", tag="ps") for _ in range(2)]
                for ct in range(8):
                    nh, cc = ct // 4, (ct % 4) * 128
                    nc.tensor.matmul(
                        vp[nh][:, cc:cc + 128],
                        vcm[ct][sl][:, PADG + pr * 128: PADG + (pr + 1) * 128],
                        id_t, start=True, stop=True)
                nc.vector.tensor_copy(vtokE[p % 8][:, 0:512], vp[0][0:64, :])
                nc.scalar.activation(vtokO[p % 8][:, 0:512],
                                     vp[0][64:128, :], AF.Copy)
                nc.vector.tensor_copy(vtokE[p % 8][:, 512:1024], vp[1][0:64, :])
                nc.scalar.activation(vtokO[p % 8][:, 512:1024],
                                     vp[1][64:128, :], AF.Copy)

            # --- depthwise conv: DVE taps into B2-seeded acc ---
            if stage < 3:
                return
            acc_t = []
            for ct in range(8):
                acc = accp.tile([128, GWP], bf16, name=f"acc{ct}", tag=f"acc{ct}")
                accv3 = acc.rearrange("p (k x) -> p k x", x=8)
                b2v3 = b2c[ct].rearrange("p (k x) -> p k x", x=8)
                srcb = vcm[ct][sl]
                for dy, dx in TAPS_DVE:
                    tap = (dy + 1) * 3 + (dx + 1)
                    dlta = 8 * dy + dx
                    xs_o = slice(max(0, -dx), 7 - max(0, dx))
                    xs_i = slice(max(0, dx) + PADG % 8, 7 - max(0, -dx) + PADG % 8)
                    src3 = bass.AP(tensor=srcb.tensor,
                                   offset=srcb.offset + PADG + 8 * dy,
                                   ap=[[srcb.ap[0][0], 128], [8, 64], [1, 8]])
                    first = (dy, dx) == TAPS_DVE[0]
                    if first:
                        # seed guard slots too (assembly never reads them,
                        # but keep the accumulate chain well-defined)
                        nc.vector.scalar_tensor_tensor(
                            out=acc, in0=srcb[:, PADG:PADG + GWP],
                            scalar=w9_t[:, ct, tap:tap + 1],
                            in1=b2c[ct], op0=ALU.mult, op1=ALU.add)
                    else:
                        nc.vector.scalar_tensor_tensor(
                            out=accv3[:, :, xs_o],
                            in0=src3[:, :, xs_o.start + dx: xs_o.stop + dx],
                            scalar=w9_t[:, ct, tap:tap + 1],
                            in1=accv3[:, :, xs_o], op0=ALU.mult, op1=ALU.add)
                acc_t.append(acc)

            # --- qk logits (rows = (hh, slot64)) ---
            if stage < 4:
                return
            Lp = [ps.tile([128, 512], f32, name="ps", tag="ps") for _ in range(4)]
            for ig in range(8):
                for h in range(H):
                    j, hh = h // 2, h % 2
                    t4, row = h // 4, (h % 4) * 32
                    nc.tensor.matmul(
                        Lp[j][64 * hh: 64 * hh + SL, ig * N:(ig + 1) * N],
                        kcm[t4][sl][row:row + 32, ig * SL:(ig + 1) * SL],
                        qcm[t4][sl][row:row + 32, ig * N:(ig + 1) * N],
                        start=True, stop=True,
                        tile_position=(row, 64 * hh))
            for j in range(4):
                with tc.high_priority(700):
                    nc.scalar.activation(Ls[j][sl], Lp[j][:, 0:GW], AF.Copy)

            # --- talking heads 1 (+ rel-pos bias) + exp ---
            if stage < 5:
                return
            E = []
            L2p = [ps.tile([128, 512], f32, name="ps", tag="ps") for _ in range(4)]
            for jo in range(4):
                for ji in range(4):
                    nc.tensor.matmul(L2p[jo][:, 0:GW],
                                     w1_t[:, jo * 4 + ji, :],
                                     Ls[ji][sl],
                                     start=(ji == 0), stop=(ji == 3))
            for jo in range(4):
                e0 = mid.tile([128, GW], bf16, name="E0", tag="E0", bufs=4)
                e = mid.tile([128, GW], bf16, name="E", tag="E", bufs=8)
                with tc.high_priority(700):
                    nc.scalar.activation(e0, L2p[jo][:, 0:GW], AF.Exp)
                    nc.vector.tensor_mul(e, e0, bias_t[jo])
                E.append(e)

            # --- softmax denominator ---
            if stage < 6:
                return
            csp = ps.tile([128, 512], f32, name="ps", tag="ps")
            for j in range(4):
                nc.tensor.matmul(csp[32 * j: 32 * j + 2, 0:GW], sel_t, E[j],
                                 start=True, stop=True,
                                 tile_position=(0, 32 * j))
            with tc.high_priority(700):
                for j in range(4):
                    nc.vector.reciprocal(r_sb[sl][32 * j: 32 * j + 2, :],
                                         csp[32 * j: 32 * j + 2, 0:GW])

            # --- normalize + talking heads 2 ---
            A = []
            for j in range(4):
                rp = ps.tile([128, 512], f32, name="ps", tag="ps")
                nc.tensor.matmul(rp[:, 0:GW], dlt_t[32 * j: 32 * j + 2, :],
                                 r_sb[sl][32 * j: 32 * j + 2, :],
                                 start=True, stop=True,
                                 tile_position=(32 * j, 0))
                a = mid.tile([128, GW], bf16, name="A", tag="A", bufs=8)
                with tc.high_priority(700):
                    nc.vector.tensor_mul(a, E[j], rp[:, 0:GW])
                A.append(a)
            A2p = [ps.tile([128, 512], f32, name="ps", tag="ps") for _ in range(4)]
            for jo in range(4):
                for ji in range(4):
                    nc.tensor.matmul(A2p[jo][:, 0:GW],
                                     w2_t[:, jo * 4 + ji, :],
                                     A[ji],
                                     start=(ji == 0), stop=(ji == 3))
            for jo in range(4):
                with tc.high_priority(700):
                    nc.scalar.activation(a2h[jo][0][sl], A2p[jo][0:64, 0:GW],
                                         AF.Copy)
                    nc.scalar.activation(a2h[jo][1][sl], A2p[jo][64:128, 0:GW],
                                         AF.Copy)

            # --- attention * V (+ PE conv taps), assembly, relu ---
            if stage < 7:
                return
            relu_t = []
            for ct in range(8):
                op2 = ps.tile([128, 512], f32, name="ps", tag="ps")
                jo, hh = ct // 2, ct % 2
                for ig in range(8):
                    i = 8 * g + ig
                    pp = ig % 2
                    vt = (vtokE if pp == 0 else vtokO)[(i // 2) % 8]
                    nc.tensor.matmul(
                        op2[:, ig * N:(ig + 1) * N],
                        vt[0:SL, ct * 128:(ct + 1) * 128],
                        a2h[jo][hh][sl][0:SL, ig * N:(ig + 1) * N],
                        start=True, stop=True)
                if stage >= 8:
                    cps = ps.tile([128, 512], f32, name="ps", tag="ps")
                    for ti, (dy, dx) in enumerate(TAPS_PE):
                        dlta = 8 * dy + dx
                        nc.tensor.matmul(
                            cps[:, 0:GWP],
                            dw_t[:, ct * len(TAPS_PE) + ti, :],
                            vcm[ct][sl][:, PADG + dlta: PADG + dlta + GWP],
                            start=(ti == 0), stop=(ti == len(TAPS_PE) - 1))
                tmp = mid.tile([128, GW], bf16, name="tmp", tag="tmp", bufs=3)
                op2d = mid.tile([128, GW], bf16, name="op2d", tag="op2d", bufs=3)
                opv = op2[:, 0:GW].rearrange("p (i y x) -> p i y x", y=7, x=7)
                nc.scalar.activation(op2d, op2[:, 0:GW], AF.Copy)
                accv = acc_t[ct].rearrange("p (i q) -> p i q", q=SL)
                accv = accv.rearrange("p i (y x) -> p i y x",
                                      x=8)[:, :, 0:7, 0:7]
                nc.vector.tensor_add(cview(tmp), cview(op2d), accv)
                if stage >= 8:
                    cpsd = mid.tile([128, GW], bf16, name="cpsd", tag="cpsd",
                                    bufs=3)
                    cpsv = cps[:, 0:GWP].rearrange("p (i q) -> p i q", q=SL)
                    cpsv = cpsv.rearrange("p i (y x) -> p i y x", x=8)[:, :, 0:7, 0:7]
                    nc.scalar.activation(cview(cpsd), cpsv, AF.Copy)
                    nc.vector.tensor_add(cview(tmp), cview(tmp), cview(cpsd))
                rl = mid.tile([128, GW], bf16, name="rl", tag="rl", bufs=10)
                nc.vector.tensor_scalar_max(rl, tmp, 0.0)
                relu_t.append(rl)

            # --- projection + store ---
            for mt in range(3):
                st = stg.tile([128, GW], bf16, name="st", tag="st")
                pp_ = ps.tile([128, 512], f32, name="ps", tag="ps")
                for kt in range(8):
                    nc.tensor.matmul(pp_[:, 0:GW],
                                     wp_t[kt][:, mt * 128:(mt + 1) * 128],
                                     relu_t[kt],
                                     start=(kt == 0), stop=(kt == 7))
                nc.scalar.activation(st, pp_[:, 0:GW], AF.Identity,
                                     bias=bp_t[:, mt:mt + 1])
                dma(out=out_d[mt][:, c0:c0 + GW], in_=st)

        if loop_n > 1:
            with tc.For_i(0, loop_n, 1):
                for g in range(NG):
                    group_body(g)
        else:
            for g in range(NG):
                group_body(g)

    nc.compile()
    return nc


_CACHE = {}


def _get_program(n_imgs):
    if n_imgs not in _CACHE:
        _CACHE[n_imgs] = build_program(n_imgs)
    return _CACHE[n_imgs]


_CONSTS_CACHE = {}


def _cached_consts(inputs):
    w = np.asarray(inputs['q_w'])
    key = (w.shape, w.dtype.str, w.tobytes()[:256])
    if key not in _CONSTS_CACHE:
        _CONSTS_CACHE.clear()
        _CONSTS_CACHE[key] = make_consts(inputs)
    return _CONSTS_CACHE[key]


def make_in_maps(inputs, n_cores=NCORES):
    """Host prep: shard + channel-major x, build replicated constants."""
    consts = _cached_consts(inputs)
    x = np.asarray(inputs['x'], np.float32)
    B = x.shape[0]
    ni = B // n_cores
    nt = ni * N
    x = x.reshape(B, N, DIM)
    in_maps = []
    for c in range(n_cores):
        m = dict(consts)
        xc = x[c * ni:(c + 1) * ni].reshape(nt, DIM).T    # [384, nt]
        m['x'] = np.ascontiguousarray(xc).reshape(3, 128, nt).astype(_BF16)
        in_maps.append(m)
    return in_maps, ni


def assemble_out(results, ni):
    """[3,128,nt] bf16 per core -> full [B, R, R, DIM] f32."""
    nt = ni * N
    outs = []
    for r in results:
        oc = np.asarray(r['out'], np.float32).reshape(DIM, nt)
        outs.append(oc.T.reshape(ni, R, R, DIM))
    return np.concatenate(outs, axis=0)


def kernel(**inputs):
    from concourse import bass_utils
    in_maps, ni = make_in_maps(inputs)
    nc = _get_program(ni)
    res = bass_utils.run_bass_kernel_spmd(
        nc, in_maps, core_ids=list(range(NCORES)))
    return assemble_out(res.results, ni).astype(np.float32)
